# revision 7
# baseline (speedup 1.0000x reference)
"""Trainium2 Bass kernel for the RPO actor (MLP encoder -> masked LSTM -> Gaussian head).

Sharding: data-parallel over the env dim B (1024 envs -> 128 per NeuronCore),
weights replicated, no collectives. Host does all layout work (transposes,
gate reorder, mask broadcast); device runs feature-major ("transposed")
matmuls + a tanh-only LSTM cell:
    sigmoid(x) = (1 + tanh(x/2)) / 2
so the four gate activations collapse into ONE tanh(0.5*gates) op per step
(g-gate weights are pre-doubled on the host), and the sigmoid affines are
folded into fused scalar_tensor_tensor ops / host-side constant scalings.
"""

import math
import numpy as np
import ml_dtypes
from contextlib import ExitStack

import concourse.bass as bass
import concourse.tile as tile
import concourse.mybir as mybir
from concourse import bacc
from concourse.bass_utils import run_bass_kernel_spmd

OBS, ACT_D = 128, 8
T, B = 256, 1024
H1, H2, HL = 512, 256, 128
NCORES = 8
BS = B // NCORES          # 128 envs per core
ROWS = T * BS             # 32768 rows per core
TC = 32                   # timesteps per chunk
NCHUNK = T // TC
G = 2                     # steps per x-matmul group (must divide TC)

BF16 = mybir.dt.bfloat16
F32 = mybir.dt.float32
NPBF16 = ml_dtypes.bfloat16
Tanh = mybir.ActivationFunctionType.Tanh
ADD = mybir.AluOpType.add
MULT = mybir.AluOpType.mult
SUB = mybir.AluOpType.subtract

_BUILD_CACHE = {}


def _build(with_bias: bool, nchunk: int = NCHUNK):
    nc = bacc.Bacc("TRN2", target_bir_lowering=False, debug=False,
                   num_devices=NCORES)
    P = lambda n, s, d, o=False: nc.declare_dram_parameter(n, s, d, isOutput=o)
    stateT = P("stateT", [OBS, ROWS], BF16)
    mhalf = P("mhalf", [128, (T + 1) * BS], BF16)   # 0.5*(1-done), bcast over partitions
    az2s = P("az2s", [BS, T * ACT_D], F32)          # (action - z - bmean)/std, env-major
    w1t = P("w1t", [OBS, H1], BF16)
    w2t = P("w2t", [H1, H2], BF16)
    wiht = P("wiht", [H2, 4 * HL], BF16)            # gate cols (i,f,o,2g)
    whht = P("whht", [HL, 4 * HL], BF16)
    biasr = P("biasr", [1, 4 * HL], BF16)           # (bih+bhh) reordered, g doubled
    wmsh = P("wmsh", [HL, ACT_D], BF16)             # (Wmean/std).T / 2
    h0T = P("h0T", [HL, BS], F32)
    c0T = P("c0T", [HL, BS], F32)
    lp_o = P("lp", [BS, T], F32, True)              # sum-of-squares, env-major
    h_o = P("h_o", [HL, BS], F32, True)
    c_o = P("c_o", [HL, BS], F32, True)

    with tile.TileContext(nc) as tc:
        with ExitStack() as ctx:
            const = ctx.enter_context(tc.tile_pool(name="const", bufs=1))
            stream = ctx.enter_context(tc.tile_pool(name="stream", bufs=2))
            xpool = ctx.enter_context(tc.tile_pool(name="xpool", bufs=4))
            h1pool = ctx.enter_context(tc.tile_pool(name="h1pool", bufs=1))
            work = ctx.enter_context(tc.tile_pool(name="work", bufs=4))
            statep = ctx.enter_context(tc.tile_pool(name="statep", bufs=3))
            mlp_ps = ctx.enter_context(
                tc.tile_pool(name="mlp_ps", bufs=1, space="PSUM"))
            gate_ps = ctx.enter_context(
                tc.tile_pool(name="gate_ps", bufs=2, space="PSUM"))
            mean_psp = ctx.enter_context(
                tc.tile_pool(name="mean_ps", bufs=2, space="PSUM"))

            # ---- load constants ----
            w1_sb = const.tile([OBS, H1], BF16, tag="w1")
            nc.gpsimd.dma_start(w1_sb[:], w1t[:])
            w2_sb = [const.tile([128, H2], BF16, tag=f"w2_{k}", name=f"w2_{k}")
                     for k in range(4)]
            for k in range(4):
                nc.gpsimd.dma_start(w2_sb[k][:], w2t[k * 128:(k + 1) * 128, :])
            wih_sb = [const.tile([128, 4 * HL], BF16, tag=f"wih_{k}", name=f"wih_{k}")
                      for k in range(2)]
            for k in range(2):
                nc.gpsimd.dma_start(wih_sb[k][:], wiht[k * 128:(k + 1) * 128, :])
            whh_sb = const.tile([HL, 4 * HL], BF16, tag="whh")
            nc.gpsimd.dma_start(whh_sb[:], whht[:])
            wms_sb = const.tile([HL, ACT_D], BF16, tag="wms")
            nc.gpsimd.dma_start(wms_sb[:], wmsh[:])
            az_sb = const.tile([BS, T, ACT_D], F32, tag="az")
            nc.gpsimd.dma_start(az_sb[:], az2s[:])
            h0_sb = const.tile([HL, BS], F32, tag="h0")
            nc.gpsimd.dma_start(h0_sb[:], h0T[:])
            c0_sb = const.tile([HL, BS], F32, tag="c0")
            nc.gpsimd.dma_start(c0_sb[:], c0T[:])
            lp_sb = const.tile([BS, T], F32, tag="lp")
            if with_bias:
                bias_sb = const.tile([1, 4 * HL], BF16, tag="bias")
                nc.gpsimd.dma_start(bias_sb[:], biasr[:])
                ones_sb = const.tile([1, G * BS], BF16, tag="ones")
                nc.vector.memset(ones_sb[:], 1.0)

            c_prev = c0_sb
            hm_prev = None  # set after mask of chunk 0 is loaded
            mean_ps = None
            Tt_last = None
            thc_last = None

            for c in range(nchunk):
                rl0 = c * TC * BS  # first row (in ROWS) of this chunk
                # ---- stream in state + mask ----
                stT = stream.tile([OBS, TC * BS], BF16, tag="stT")
                nc.gpsimd.dma_start(stT[:], stateT[:, rl0:rl0 + TC * BS])
                mh = stream.tile([128, (TC + 1) * BS], BF16, tag="mh")
                nc.gpsimd.dma_start(mh[:], mhalf[:, rl0:rl0 + (TC + 1) * BS])

                if c == 0:
                    # hm_0 = (2*h0) * mhalf[0]  (= h0 * mask)
                    hm0 = work.tile([HL, BS], BF16, tag="hm")
                    nc.vector.scalar_tensor_tensor(
                        hm0[:], h0_sb[:], 2.0, mh[:, 0:BS], op0=MULT, op1=MULT)
                    hm_prev = hm0

                # ---- MLP layer 1: h1T[j] = tanh(W1_j @ stateT) ----
                h1t = h1pool.tile([128, 4, TC * BS], BF16, tag="h1t")
                NQ = TC * BS // 1024
                for j in range(4):
                    for q in range(NQ):
                        ps = mlp_ps.tile([128, 1024], F32, tag="mlp")
                        for s in range(2):
                            nc.tensor.matmul(
                                ps[:, s * 512:(s + 1) * 512],
                                w1_sb[:, j * 128:(j + 1) * 128],
                                stT[:, q * 1024 + s * 512:q * 1024 + (s + 1) * 512],
                                start=True, stop=True)
                        nc.scalar.activation(
                            h1t[:, j, q * 1024:(q + 1) * 1024], ps[:], Tanh)

                # ---- MLP layer 2: xT[jj] = tanh(W2_jj @ h1T) ----
                xT = [xpool.tile([128, TC * BS], BF16, tag=f"xt{jj}", name=f"xt{jj}")
                      for jj in range(2)]
                for jj in range(2):
                    for q in range(NQ):
                        ps = mlp_ps.tile([128, 1024], F32, tag="mlp")
                        for s in range(2):
                            lo = q * 1024 + s * 512
                            for k in range(4):
                                nc.tensor.matmul(
                                    ps[:, s * 512:(s + 1) * 512],
                                    w2_sb[k][:, jj * 128:(jj + 1) * 128],
                                    h1t[:, k, lo:lo + 512],
                                    start=(k == 0), stop=(k == 3))
                        nc.scalar.activation(
                            xT[jj][:, q * 1024:(q + 1) * 1024], ps[:], Tanh)

                # ---- LSTM scan over this chunk ----
                for tl0 in range(0, TC, G):
                    # gates psum: [feat, gate j, (dt, b)] ; x-part batched over G steps
                    # One accumulation group per PSUM bank: start=True only on
                    # the first matmul touching the bank (it clears has_written
                    # for the WHOLE bank), stop=True on the bank's last matmul
                    # (the final recurrent matmul below). With G=2, gates (i,f)
                    # share bank 0 and (o,g) share bank 1.
                    ps = gate_ps.tile([128, 4, G * BS], F32, tag="gates")
                    for j in range(4):
                        for k in range(2):
                            nc.tensor.matmul(
                                ps[:, j, :],
                                wih_sb[k][:, j * 128:(j + 1) * 128],
                                xT[k][:, tl0 * BS:(tl0 + G) * BS],
                                start=(k == 0 and j % 2 == 0), stop=False,
                                skip_group_check=True)
                        if with_bias:
                            nc.tensor.matmul(
                                ps[:, j, :],
                                bias_sb[:, j * 128:(j + 1) * 128],
                                ones_sb[:],
                                start=False, stop=False,
                                skip_group_check=True)
                    for dt in range(G):
                        tl = tl0 + dt
                        t = c * TC + tl
                        # masked cell state (cm = c * mask/2)
                        cm = work.tile([HL, BS], F32, tag="cm")
                        nc.vector.tensor_tensor(
                            cm[:], c_prev[:], mh[:, tl * BS:(tl + 1) * BS], op=MULT)
                        # recurrent matmuls
                        for j in range(4):
                            nc.tensor.matmul(
                                ps[:, j, dt * BS:(dt + 1) * BS],
                                whh_sb[:, j * 128:(j + 1) * 128],
                                hm_prev[:],
                                start=False,
                                stop=(dt == G - 1 and j % 2 == 1),
                                skip_group_check=True)
                        # one tanh for all gates: T = tanh(0.5 * gates)
                        Tt = work.tile([128, 4, BS], BF16, tag="Tt")
                        nc.scalar.activation(
                            Tt[:], ps[:, 0:4, dt * BS:(dt + 1) * BS], Tanh,
                            scale=0.5)
                        Ti, Tf, To, Tg = (Tt[:, 0, :], Tt[:, 1, :],
                                          Tt[:, 2, :], Tt[:, 3, :])
                        # c_new = (1+Tf)*cm + 0.5*(1+Ti)*Tg
                        t2x = work.tile([HL, BS], BF16, tag="t2x")
                        nc.vector.scalar_tensor_tensor(
                            t2x[:], Ti, 1.0, Tg, op0=ADD, op1=MULT)
                        t1 = work.tile([HL, BS], F32, tag="t1")
                        nc.vector.scalar_tensor_tensor(
                            t1[:], Tf, 1.0, cm[:], op0=ADD, op1=MULT)
                        c_new = statep.tile([HL, BS], F32, tag="c")
                        nc.vector.scalar_tensor_tensor(
                            c_new[:], t2x[:], 0.5, t1[:], op0=MULT, op1=ADD)
                        thc = work.tile([HL, BS], BF16, tag="thc")
                        nc.scalar.activation(thc[:], c_new[:], Tanh)
                        # om = (1+To) * mask/2 ; hm = om * thc ( = sig(o)*mask*tanh(c))
                        om = work.tile([HL, BS], BF16, tag="om")
                        nc.vector.scalar_tensor_tensor(
                            om[:], To, 1.0, mh[:, (tl + 1) * BS:(tl + 2) * BS],
                            op0=ADD, op1=MULT)
                        hm = work.tile([HL, BS], BF16, tag="hm")
                        nc.vector.tensor_tensor(hm[:], om[:], thc[:], op=MULT)
                        # hp = (1+To)*thc = 2*h  (head lhsT; Wmean pre-halved)
                        hp = work.tile([HL, BS], BF16, tag="hp")
                        nc.vector.scalar_tensor_tensor(
                            hp[:], To, 1.0, thc[:], op0=ADD, op1=MULT)
                        # head matmul: mean/std accumulated per 64-step epoch
                        tmod = t % 64
                        if tmod == 0:
                            mean_ps = mean_psp.tile([BS, 64, ACT_D], F32,
                                                    tag="mean")
                        nc.tensor.matmul(
                            mean_ps[:, tmod, :], hp[:], wms_sb[:],
                            start=True, stop=True)
                        if tmod == 63:
                            e = t // 64
                            diff = work.tile([BS, 64, ACT_D], F32, tag="diff")
                            nc.vector.tensor_tensor(
                                diff[:], az_sb[:, e * 64:(e + 1) * 64, :],
                                mean_ps[:], op=SUB)
                            sq = work.tile([BS, 64, ACT_D], BF16, tag="sq")
                            nc.vector.tensor_tensor(
                                sq[:], diff[:], diff[:], op=MULT)
                            nc.vector.tensor_reduce(
                                lp_sb[:, e * 64:(e + 1) * 64], sq[:],
                                axis=mybir.AxisListType.X, op=ADD)
                        c_prev = c_new
                        hm_prev = hm
                        if t == nchunk * TC - 1:
                            Tt_last, thc_last = Tt, thc

            # ---- final outputs ----
            # h_T = 0.5*(1+To)*tanh(c_T), recomputed in f32 off the last step
            thc32 = work.tile([HL, BS], F32, tag="thc32")
            nc.scalar.activation(thc32[:], c_prev[:], Tanh)
            hf1 = work.tile([HL, BS], F32, tag="hf1")
            nc.vector.scalar_tensor_tensor(
                hf1[:], Tt_last[:, 2, :], 1.0, thc32[:], op0=ADD, op1=MULT)
            hf = work.tile([HL, BS], F32, tag="hf")
            nc.vector.tensor_scalar_mul(hf[:], hf1[:], 0.5)
            nc.gpsimd.dma_start(h_o[:], hf[:])
            nc.gpsimd.dma_start(c_o[:], c_prev[:])
            ncol = max(64, (nchunk * TC // 64) * 64)
            nc.gpsimd.dma_start(lp_o[:, 0:ncol], lp_sb[:, 0:ncol])

    nc.finalize()
    return nc


def get_nc(with_bias: bool, nchunk: int = NCHUNK):
    key = (bool(with_bias), nchunk)
    if key not in _BUILD_CACHE:
        _BUILD_CACHE[key] = _build(with_bias, nchunk)
    return _BUILD_CACHE[key]


def _prep_core_inputs(inputs):
    """Host-side layout prep. Returns (in_maps, host_ctx)."""
    state = np.asarray(inputs["state"], np.float32)
    done = np.asarray(inputs["done"], np.float32)
    h0 = np.asarray(inputs["h0"], np.float32)
    c0 = np.asarray(inputs["c0"], np.float32)
    action = np.asarray(inputs["action"], np.float32)
    z = np.asarray(inputs["z"], np.float32)
    W1 = np.asarray(inputs["W1"], np.float32)
    W2 = np.asarray(inputs["W2"], np.float32)
    Wih = np.asarray(inputs["Wih"], np.float32)
    Whh = np.asarray(inputs["Whh"], np.float32)
    bih = np.asarray(inputs["bih"], np.float32)
    bhh = np.asarray(inputs["bhh"], np.float32)
    Wmean = np.asarray(inputs["Wmean"], np.float32)
    bmean = np.asarray(inputs["bmean"], np.float32)
    logstd = np.asarray(inputs["logstd"], np.float32)

    std = np.exp(logstd[0])                      # [8]
    perm = [0, 1, 3, 2]                          # torch (i,f,g,o) -> (i,f,o,g)
    gscale = np.array([1.0, 1.0, 1.0, 2.0], np.float32)[:, None]

    wih_r = Wih.reshape(4, HL, H2)[perm] * gscale[:, :, None]
    whh_r = Whh.reshape(4, HL, HL)[perm] * gscale[:, :, None]
    bias_r = ((bih + bhh).reshape(4, HL)[perm] * gscale).reshape(1, 4 * HL)
    with_bias = bool(np.any(bias_r != 0.0))

    wiht = np.ascontiguousarray(wih_r.reshape(4 * HL, H2).T).astype(NPBF16)
    whht = np.ascontiguousarray(whh_r.reshape(4 * HL, HL).T).astype(NPBF16)
    biasr = bias_r.astype(NPBF16)
    w1t = np.ascontiguousarray(W1.T).astype(NPBF16)         # [OBS, H1]
    w2t = np.ascontiguousarray(W2.T).astype(NPBF16)         # [H1, H2]
    wmsh = np.ascontiguousarray((Wmean / (2.0 * std[:, None])).T).astype(NPBF16)

    st = state.reshape(T, B, OBS)
    dn = done.reshape(T, B)
    ac = action.reshape(T, B, ACT_D)
    zz = z.reshape(T, B, ACT_D)

    in_maps = []
    for core in range(NCORES):
        sl = slice(core * BS, (core + 1) * BS)
        stateT = np.ascontiguousarray(
            st[:, sl, :].transpose(2, 0, 1).reshape(OBS, ROWS)).astype(NPBF16)
        mrow = 0.5 * (1.0 - dn[:, sl]).reshape(ROWS)
        mrow = np.concatenate([mrow, np.full(BS, 0.5, np.float32)])
        mhalf = np.ascontiguousarray(
            np.broadcast_to(mrow[None, :], (128, ROWS + BS))).astype(NPBF16)
        az = (ac[:, sl, :] - zz[:, sl, :] - bmean[None, None, :]) / std
        az2s = np.ascontiguousarray(
            az.transpose(1, 0, 2).reshape(BS, T * ACT_D)).astype(np.float32)
        in_maps.append({
            "stateT": stateT, "mhalf": mhalf, "az2s": az2s,
            "w1t": w1t, "w2t": w2t, "wiht": wiht, "whht": whht,
            "biasr": biasr, "wmsh": wmsh,
            "h0T": np.ascontiguousarray(h0[0, sl, :].T).astype(np.float32),
            "c0T": np.ascontiguousarray(c0[0, sl, :].T).astype(np.float32),
        })
    host_ctx = dict(action=inputs["action"], logstd=logstd, with_bias=with_bias)
    return in_maps, host_ctx


def _assemble(results, host_ctx):
    logstd = host_ctx["logstd"]
    LOG2PI = math.log(2.0 * math.pi)
    const = float(np.sum(logstd) + 0.5 * ACT_D * LOG2PI)
    lp = np.empty((T, B), np.float32)
    hT = np.empty((1, B, HL), np.float32)
    cT = np.empty((1, B, HL), np.float32)
    for core in range(NCORES):
        sl = slice(core * BS, (core + 1) * BS)
        r = results[core]
        lp[:, sl] = r["lp"].T
        hT[0, sl, :] = r["h_o"].T
        cT[0, sl, :] = r["c_o"].T
    logprob = (-0.5 * lp.reshape(T * B)) - const
    ent_row = float(np.sum(0.5 + 0.5 * LOG2PI + logstd))
    entropy = np.full(T * B, ent_row, np.float32)
    action = np.asarray(host_ctx["action"], np.float32)
    return action, logprob.astype(np.float32), entropy, hT, cT


def kernel(**inputs):
    in_maps, host_ctx = _prep_core_inputs(inputs)
    nc = get_nc(host_ctx["with_bias"])
    res = run_bass_kernel_spmd(nc, in_maps, core_ids=list(range(NCORES)))
    return _assemble(res.results, host_ctx)
